# revision 1
# baseline (speedup 1.0000x reference)
"""DenseEdgeConv (gnn_message_passing) Trainium2 Bass kernel.

Problem: B=8 point clouds of N=4096 points. Per cloud: exact 16-NN by
Euclidean distance (excluding self), gather neighbor features, edge MLP,
channel gate, max-aggregation.  Output (B, N, 160) fp32.

Strategy: batch-parallel over 8 NeuronCores (1 cloud/core), no collectives.

Per-core algorithm (all layouts "feature-major" = channels on partitions,
points/edges on the free axis, so matmuls chain on the PE without
transposes):

 1. Ranking matmul: val[i,j] = 2 p_i.p_j - |p_j|^2  (= -dist + const(i));
    self is always the row max, excluded by writing -BIG on the diagonal
    (gpsimd affine_select).
 2. Exact top-16 per row with the DVE max8/max_index/match_replace ISA:
    5 linear scans per 128-row tile.
 3. Neighbor gather with 16 indirect DMAs (one per neighbor rank; edges are
    ordered k-major so the offset columns are exactly the max_index outputs).
 4. Edge MLP with the first layer factored:
       relu(edge @ W1) = relu(x_i @ (W1a-W1c) + x_j @ (W1b+W1c))
    The x_i "broadcast over 16 neighbors" terms are injected via a second
    accumulating matmul against a constant 0/1 expansion matrix E
    (E[i, e] = 1 iff e//16 == i), so no elementwise broadcast is needed.
 5. Gate/aggregation algebra: max_k(y*gate) = gate*max_k(y) (gate>0), the
    x-channels of y are constant over k so their pooled value is just
    gate*x, and blast is folded in after the max-pool.
"""

import os
import sys

sys.path.insert(0, "/opt/trn_rl_repo")

import numpy as np

import concourse.bass as bass
import concourse.bacc as bacc
import concourse.tile as tile
from concourse import mybir
from concourse.bass_utils import run_bass_kernel_spmd

F32 = mybir.dt.float32
BF16 = mybir.dt.bfloat16
U32 = mybir.dt.uint32
I16 = mybir.dt.int16

B, N, D, G, K = 8, 4096, 64, 32, 16
COUT = D + 3 * G  # 160
NT = N // 128     # 32 row tiles
NEG = -3.0e38
AF = mybir.ActivationFunctionType
ALU = mybir.AluOpType


def build_nc(finalize: bool = True) -> bass.Bass:
    # Bacc (not plain Bass): its compile pass handles register allocation
    # and event-semaphore fusion that walrus codegen requires.
    nc = bacc.Bacc()

    # ---- DRAM parameters (per-core inputs) ----
    x_d = nc.dram_tensor("x", [N, D], F32, kind="ExternalInput")
    L_d = nc.dram_tensor("Lm", [4, N], F32, kind="ExternalInput")    # [2 p^T; -1]
    R_d = nc.dram_tensor("Rm", [4, N], F32, kind="ExternalInput")    # [p^T; |p|^2]
    AR_d = nc.dram_tensor("AR", [D, 4 * D + G], F32, kind="ExternalInput")  # [A | Wm_x]
    Bm_d = nc.dram_tensor("Bmat", [D, 4 * D], BF16, kind="ExternalInput")
    W2_d = nc.dram_tensor("W2", [4 * D, G], F32, kind="ExternalInput")
    Wmh_d = nc.dram_tensor("Wmh", [G, G], F32, kind="ExternalInput")
    Wg_d = nc.dram_tensor("Wg", [D + 2 * G, D + 2 * G], F32, kind="ExternalInput")
    Wl_d = nc.dram_tensor("Wlast", [D + 2 * G, G], F32, kind="ExternalInput")
    E_d = nc.dram_tensor("Em", [128, 128 * K], BF16, kind="ExternalInput")
    id_d = nc.dram_tensor("ident", [128, 128], F32, kind="ExternalInput")
    b1_d = nc.dram_tensor("b1", [128, 2], F32, kind="ExternalInput")
    b2_d = nc.dram_tensor("b2", [G, 1], F32, kind="ExternalInput")
    bmid_d = nc.dram_tensor("bmid", [G, 1], F32, kind="ExternalInput")
    bg_d = nc.dram_tensor("bg", [D + 2 * G, 1], F32, kind="ExternalInput")
    blast_d = nc.dram_tensor("blast", [G, 1], F32, kind="ExternalInput")
    out_d = nc.dram_tensor("out", [N, COUT], F32, kind="ExternalOutput")

    E_COLS = 128 * K  # 2048 edges per row-tile
    NCH = 4           # edge chunks per row-tile
    CH = E_COLS // NCH  # 512

    with tile.TileContext(nc) as tc:
        with (
            tc.tile_pool(name="singles", bufs=1) as singles,
            tc.tile_pool(name="vals", bufs=2) as vals,
            tc.tile_pool(name="acts", bufs=2) as acts,
            tc.tile_pool(name="small", bufs=3) as small,
            tc.tile_pool(name="outs", bufs=2) as outs,
            tc.tile_pool(name="ps_val", bufs=2, space="PSUM") as ps_val,
            tc.tile_pool(name="ps_h1", bufs=2, space="PSUM") as ps_h1,
            tc.tile_pool(name="ps_a", bufs=2, space="PSUM") as ps_a,
            tc.tile_pool(name="ps_b", bufs=2, space="PSUM") as ps_b,
        ):
            # ---- load constants / weights into SBUF once ----
            R_sb = singles.tile([4, N], F32)
            nc.sync.dma_start(out=R_sb, in_=R_d[:, :])
            E_sb = singles.tile([128, E_COLS], BF16)
            nc.sync.dma_start(out=E_sb, in_=E_d[:, :])
            id_sb = singles.tile([128, 128], F32)
            nc.sync.dma_start(out=id_sb, in_=id_d[:, :])
            AR_sb = singles.tile([D, 4 * D + G], F32)
            nc.sync.dma_start(out=AR_sb, in_=AR_d[:, :])
            Bm_sb = singles.tile([D, 4 * D], BF16)
            nc.sync.dma_start(out=Bm_sb, in_=Bm_d[:, :])
            W2a_sb = singles.tile([128, G], F32)
            nc.sync.dma_start(out=W2a_sb, in_=W2_d[0:128, :])
            W2b_sb = singles.tile([128, G], F32)
            nc.sync.dma_start(out=W2b_sb, in_=W2_d[128:256, :])
            # Wmh sits at partition base 32 so its matmul rhs (yfm[32:64])
            # has a matching base partition.
            Wmh_sb = singles.tile([2 * G, G], F32)
            nc.sync.dma_start(out=Wmh_sb[G:2 * G, :], in_=Wmh_d[:, :])
            Wg_sb = singles.tile([128, 128], F32)
            nc.sync.dma_start(out=Wg_sb, in_=Wg_d[:, :])
            Wl_sb = singles.tile([128, G], F32)
            nc.sync.dma_start(out=Wl_sb, in_=Wl_d[:, :])
            # rows 64:128 of Wlast again at base partition 0 (gxw matmul rhs)
            Wl2_sb = singles.tile([D, G], F32)
            nc.sync.dma_start(out=Wl2_sb, in_=Wl_d[2 * G:128, :])
            b1_sb = singles.tile([128, 2], F32)
            nc.sync.dma_start(out=b1_sb, in_=b1_d[:, :])
            b2_sb = singles.tile([G, 1], F32)
            nc.sync.dma_start(out=b2_sb, in_=b2_d[:, :])
            bmid_sb = singles.tile([G, 1], F32)
            nc.sync.dma_start(out=bmid_sb, in_=bmid_d[:, :])
            bg_sb = singles.tile([128, 1], F32)
            nc.sync.dma_start(out=bg_sb, in_=bg_d[:, :])
            blast_sb = singles.tile([G, 1], F32)
            nc.sync.dma_start(out=blast_sb, in_=blast_d[:, :])

            # one-time gpsimd register (to_reg per call exhausts the file)
            neg_reg = nc.gpsimd.to_reg(NEG)

            for t in range(NT):
                r0 = 128 * t

                # ---------- ranking matmul: val = L_t^T @ R ----------
                L_sb = small.tile([4, 128], F32, tag="ltile")
                nc.sync.dma_start(out=L_sb, in_=L_d[:, r0:r0 + 128])
                val_sb = vals.tile([128, N], F32, tag="val")
                for q in range(N // 512):
                    vps = ps_val.tile([128, 512], F32, tag="vps")
                    nc.tensor.matmul(vps, L_sb, R_sb[:, 512 * q:512 * (q + 1)],
                                     start=True, stop=True)
                    nc.scalar.copy(out=val_sb[:, 512 * q:512 * (q + 1)], in_=vps)

                # exclude self: val[r, r0+r] = -BIG (iota = j - p over the
                # diagonal 128-col block)
                nc.gpsimd.affine_select(
                    out=val_sb[:, r0:r0 + 128], in_=val_sb[:, r0:r0 + 128],
                    pattern=[[1, 128]], compare_op=ALU.not_equal, fill=neg_reg,
                    base=0, channel_multiplier=-1)

                # ---------- top-16 (max8 x2 rounds) ----------
                m1 = small.tile([128, 8], F32, tag="m1")
                i1 = small.tile([128, 8], U32, tag="i1")
                m2 = small.tile([128, 8], F32, tag="m2")
                i2 = small.tile([128, 8], U32, tag="i2")
                nc.vector.max(out=m1, in_=val_sb)
                nc.vector.max_index(out=i1, in_max=m1, in_values=val_sb)
                nc.vector.match_replace(out=val_sb, in_to_replace=m1,
                                        in_values=val_sb, imm_value=NEG)
                nc.vector.max(out=m2, in_=val_sb)
                nc.vector.max_index(out=i2, in_max=m2, in_values=val_sb)

                # ---------- gather neighbor features (HBM row gather) ----------
                # edges are k-major: block b holds the b-th nearest neighbor
                # of all 128 points, so the offsets are columns of i1/i2.
                # NOTE: one DMA per neighbor rank — batching all 16 into one
                # indirect DMA with a (128,16) offset tensor produces wrong
                # results on HW (walrus pairs offsets with dest rows in a
                # different order than the simulator).
                xg_sb = acts.tile([128, K, D], F32, tag="xg")
                for b in range(K):
                    col = i1[:, b:b + 1] if b < 8 else i2[:, b - 8:b - 7]
                    nc.gpsimd.indirect_dma_start(
                        out=xg_sb[:, b, :], out_offset=None, in_=x_d[:, :],
                        in_offset=bass.IndirectOffsetOnAxis(ap=col, axis=0))

                # ---------- per-tile point-major x, P/R precompute ----------
                x_pm = small.tile([128, D], F32, tag="x_pm")
                nc.sync.dma_start(out=x_pm, in_=x_d[r0:r0 + 128, :])
                xT_ps = ps_b.tile([D, 128], F32, tag="psB")
                nc.tensor.transpose(xT_ps, x_pm, id_sb)
                xT_sb = small.tile([D, 128], F32, tag="xT")
                nc.scalar.copy(out=xT_sb, in_=xT_ps)

                PR_ps = ps_b.tile([128, 4 * D + G], F32, tag="psB")
                nc.tensor.matmul(PR_ps, xT_sb, AR_sb, start=True, stop=True)
                # bf16: lhsT of the E-expansion matmuls (pairs with bf16 E)
                PR_sb = small.tile([128, 4 * D + G], BF16, tag="PR")
                nc.scalar.copy(out=PR_sb, in_=PR_ps)

                # ---------- edge MLP ----------
                h1a = acts.tile([128, E_COLS], F32, tag="h1a")
                h1b = acts.tile([128, E_COLS], F32, tag="h1b")
                yfm = acts.tile([2 * G, E_COLS], F32, tag="yfm")  # [m; h2]
                for c in range(NCH):
                    ec = slice(CH * c, CH * (c + 1))
                    # transpose gathered x into feature-major (64, 512)
                    xgT_ps = ps_b.tile([D, CH], F32, tag="psB")
                    for bk in range(CH // 128):
                        nc.tensor.transpose(
                            xgT_ps[:, 128 * bk:128 * (bk + 1)],
                            xg_sb[:, (CH // 128) * c + bk, :], id_sb)
                    xgT = small.tile([D, CH], BF16, tag="xgT")
                    nc.scalar.copy(out=xgT, in_=xgT_ps)

                    # h1 = relu(Bm^T x_j + P_i + b1), two 128-ch halves
                    for h, h1_sb in ((0, h1a), (1, h1b)):
                        hps = ps_h1.tile([128, CH], F32, tag="h1ps")
                        nc.tensor.matmul(hps, Bm_sb[:, 128 * h:128 * (h + 1)],
                                         xgT, start=True, stop=False)
                        nc.tensor.matmul(hps, PR_sb[:, 128 * h:128 * (h + 1)],
                                         E_sb[:, ec], start=False, stop=True)
                        nc.scalar.activation(out=h1_sb[:, ec], in_=hps,
                                             func=AF.Relu,
                                             bias=b1_sb[:, h:h + 1])

                    # h2 = relu(W2^T h1 + b2) -> yfm rows 32:64
                    h2ps = ps_a.tile([G, CH], F32, tag="psA")
                    nc.tensor.matmul(h2ps, W2a_sb, h1a[:, ec], start=True, stop=False)
                    nc.tensor.matmul(h2ps, W2b_sb, h1b[:, ec], start=False, stop=True)
                    nc.scalar.activation(out=yfm[G:2 * G, ec], in_=h2ps,
                                         func=AF.Relu, bias=b2_sb)

                    # m = relu(Wmh^T h2 + R_i + bmid) -> yfm rows 0:32
                    mps = ps_a.tile([G, CH], F32, tag="psA")
                    nc.tensor.matmul(mps, Wmh_sb[G:2 * G, :], yfm[G:2 * G, ec],
                                     start=True, stop=False)
                    nc.tensor.matmul(mps, PR_sb[:, 4 * D:4 * D + G],
                                     E_sb[:, ec], start=False, stop=True)
                    nc.scalar.activation(out=yfm[0:G, ec], in_=mps,
                                         func=AF.Relu, bias=bmid_sb)

                # ---------- gate ----------
                # k-major edge order: position e = 128*k + point
                ymean = small.tile([128, 128], F32, tag="ymean")
                nc.vector.tensor_reduce(
                    out=ymean[0:2 * G, :],
                    in_=yfm.rearrange("p (k n) -> p n k", k=K),
                    axis=mybir.AxisListType.X, op=ALU.add)
                nc.scalar.copy(out=ymean[2 * G:128, :], in_=xT_sb)

                gps = ps_b.tile([128, 128], F32, tag="psB")
                nc.tensor.matmul(gps, Wg_sb, ymean, start=True, stop=True)
                gate_fm = small.tile([128, 128], F32, tag="gate_fm")
                nc.scalar.activation(out=gate_fm, in_=gps, func=AF.Sigmoid,
                                     bias=bg_sb)
                # gate rows 64:128 again at base partition 0: the gx multiply
                # needs both SBUF inputs on the same base partition
                gate_hi = small.tile([D, 128], F32, tag="gate_hi")
                nc.scalar.activation(out=gate_hi, in_=gps[2 * G:128, :],
                                     func=AF.Sigmoid, bias=bg_sb[2 * G:128, :])
                gpm_ps = ps_b.tile([128, 128], F32, tag="psB")
                nc.tensor.transpose(gpm_ps, gate_fm, id_sb)
                gate_pm = small.tile([128, 128], BF16, tag="gate_pm")
                nc.scalar.copy(out=gate_pm, in_=gpm_ps)

                # gx = gate[64:128] * x   (x-channels of y*gate, constant in k)
                gx_fm = small.tile([D, 128], F32, tag="gx_fm")
                nc.vector.tensor_mul(gx_fm, gate_hi, xT_sb)
                gxw_ps = ps_b.tile([128, G], F32, tag="psB")
                nc.tensor.matmul(gxw_ps, gx_fm, Wl2_sb,
                                 start=True, stop=True)
                gxw_sb = small.tile([128, G], BF16, tag="gxw")
                nc.scalar.copy(out=gxw_sb, in_=gxw_ps)

                # ---------- gated last layer + max pooling ----------
                # each 512-edge chunk covers 4 neighbor ranks of all 128
                # points; keep a running max across chunks.
                zp_sb = small.tile([G, 128], F32, tag="zp")
                for c in range(NCH):
                    ec = slice(CH * c, CH * (c + 1))
                    ggps = ps_b.tile([2 * G, CH], F32, tag="psB")
                    nc.tensor.matmul(ggps, gate_pm[:, 0:2 * G], E_sb[:, ec],
                                     start=True, stop=True)
                    # yg = (gate broadcast) * yfm — ACT drains psum, the
                    # multiply runs on the otherwise-idle gpsimd (keeps the
                    # DVE free for the top-k scans)
                    gg_sb = small.tile([2 * G, CH], F32, tag="gg")
                    nc.scalar.copy(out=gg_sb, in_=ggps)
                    yg_sb = small.tile([2 * G, CH], F32, tag="yg")
                    nc.gpsimd.tensor_tensor(out=yg_sb, in0=gg_sb,
                                            in1=yfm[:, ec], op=ALU.mult)

                    zps = ps_a.tile([G, CH], F32, tag="psA")
                    nc.tensor.matmul(zps, Wl_sb[0:2 * G, :], yg_sb,
                                     start=True, stop=False)
                    nc.tensor.matmul(zps, gxw_sb, E_sb[:, ec],
                                     start=False, stop=True)
                    ztmp = small.tile([G, 128], F32, tag="ztmp")
                    nc.vector.tensor_reduce(
                        out=ztmp,
                        in_=zps.rearrange("p (k n) -> p n k", k=CH // 128),
                        axis=mybir.AxisListType.X, op=ALU.max)
                    if c == 0:
                        nc.vector.tensor_copy(zp_sb, ztmp)
                    else:
                        nc.vector.tensor_tensor(out=zp_sb, in0=zp_sb,
                                                in1=ztmp, op=ALU.max)

                ymax = small.tile([2 * G, 128], F32, tag="ymax")
                nc.vector.tensor_reduce(
                    out=ymax, in_=yfm.rearrange("p (k n) -> p n k", k=K),
                    axis=mybir.AxisListType.X, op=ALU.max)

                # ---------- assemble output (transpose to point-major) ----------
                zb_sb = small.tile([G, 128], F32, tag="zb")
                nc.vector.tensor_add(zb_sb, zp_sb,
                                     blast_sb.to_broadcast([G, 128]))
                yout = small.tile([128, 128], F32, tag="yout")
                nc.vector.tensor_mul(yout[0:2 * G, :], gate_fm[0:2 * G, :], ymax)
                nc.scalar.copy(out=yout[2 * G:128, :], in_=gx_fm)

                zt_ps = ps_b.tile([128, G], F32, tag="psB")
                nc.tensor.transpose(zt_ps, zb_sb, id_sb[0:G, 0:G])
                zt_sb = outs.tile([128, G], F32, tag="zt")
                nc.scalar.copy(out=zt_sb, in_=zt_ps)
                nc.sync.dma_start(out=out_d[r0:r0 + 128, 0:G], in_=zt_sb)

                yt_ps = ps_b.tile([128, 128], F32, tag="psB")
                nc.tensor.transpose(yt_ps, yout, id_sb)
                yt_sb = outs.tile([128, 128], F32, tag="yt")
                nc.scalar.copy(out=yt_sb, in_=yt_ps)
                nc.sync.dma_start(out=out_d[r0:r0 + 128, G:COUT], in_=yt_sb)

    if finalize:
        nc.finalize()   # Bacc.compile: reg alloc, event sems, library loads
    return nc


_NC_CACHE = {}


def _get_nc():
    if "nc" not in _NC_CACHE:
        _NC_CACHE["nc"] = build_nc()
    return _NC_CACHE["nc"]


def _host_prep(inputs):
    """Shared (replicated) weight-derived arrays."""
    W1 = np.asarray(inputs["W1"], np.float32)
    Wmid = np.asarray(inputs["Wmid"], np.float32)
    A = W1[0:D] - W1[2 * D:3 * D]
    Bm = W1[D:2 * D] + W1[2 * D:3 * D]
    AR = np.concatenate([A, Wmid[G:G + D]], axis=1)          # (64, 288)
    Wg_adj = np.asarray(inputs["Wg"], np.float32).copy()
    Wg_adj[0:2 * G] /= K
    # k-major edge order: E[i, 128*k + p] = (p == i)
    E = np.tile(np.eye(128, dtype=np.float32), (1, K))
    ident = np.eye(128, dtype=np.float32)
    import ml_dtypes
    rep = {
        "AR": np.ascontiguousarray(AR),
        "Bmat": np.ascontiguousarray(Bm).astype(ml_dtypes.bfloat16),
        "W2": np.asarray(inputs["W2"], np.float32),
        "Wmh": np.ascontiguousarray(Wmid[0:G]),
        "Wg": Wg_adj,
        "Wlast": np.asarray(inputs["Wlast"], np.float32),
        "Em": E.astype(ml_dtypes.bfloat16),
        "ident": ident,
        "b1": np.ascontiguousarray(
            np.asarray(inputs["b1"], np.float32).reshape(2, 128).T),
        "b2": np.asarray(inputs["b2"], np.float32).reshape(G, 1),
        "bmid": np.asarray(inputs["bmid"], np.float32).reshape(G, 1),
        "bg": np.asarray(inputs["bg"], np.float32).reshape(128, 1),
        "blast": np.asarray(inputs["blast"], np.float32).reshape(G, 1),
    }
    return rep


def make_in_maps(inputs):
    x = np.asarray(inputs["x"], np.float32)
    pos = np.asarray(inputs["pos"], np.float32)
    rep = _host_prep(inputs)
    in_maps = []
    for c in range(B):
        p = pos[c]
        sq = (p * p).sum(-1)
        L = np.concatenate([2.0 * p.T, -np.ones((1, N), np.float32)], axis=0)
        R = np.concatenate([p.T, sq[None, :]], axis=0)
        m = dict(rep)
        m["x"] = np.ascontiguousarray(x[c])
        m["Lm"] = np.ascontiguousarray(L.astype(np.float32))
        m["Rm"] = np.ascontiguousarray(R.astype(np.float32))
        in_maps.append(m)
    return in_maps


def kernel(**inputs) -> np.ndarray:
    nc = _get_nc()
    in_maps = make_in_maps(inputs)
    res = run_bass_kernel_spmd(nc, in_maps, list(range(B)))
    return np.stack([res.results[c]["out"] for c in range(B)]).astype(np.float32)


if __name__ == "__main__":
    nc = build_nc()
    print("built ok:",
          sum(len(bb.instructions) for bb in nc.main_func.blocks), "instructions")



# revision 4
# speedup vs baseline: 1.4523x; 1.4523x over previous
"""DenseEdgeConv (gnn_message_passing) Trainium2 Bass kernel.

Problem: B=8 point clouds of N=4096 points. Per cloud: exact 16-NN by
Euclidean distance (excluding self), gather neighbor features, edge MLP,
channel gate, max-aggregation.  Output (B, N, 160) fp32.

Strategy: batch-parallel over 8 NeuronCores (1 cloud/core), no collectives.

The metric (wall time of a full dispatch) is transfer-dominated under the
axon PJRT tunnel, so the kernel minimizes per-call bytes:
 - x and all MLP weights ship as bf16 (the edge MLP already ran in bf16).
 - The 0/1 expansion matrix E, the transpose identity, and the ranking lhs
   L = [2p; -1] are generated on device instead of uploaded.
 - The output lands in DRAM as bf16 feature-major [160, N]; the host
   transposes and upcasts. This halves both the donated zero-output upload
   and the result download.
Ranking (distance matmul + top-k) stays fp32 end-to-end — neighbor
selection is the dominant error source and gets no dtype cut.

Per-core algorithm (all layouts "feature-major" = channels on partitions,
points/edges on the free axis, so matmuls chain on the PE without
transposes):

 1. Ranking matmul: val[i,j] = 2 p_i.p_j - |p_j|^2  (= -dist + const(i));
    self is always the row max, excluded by writing -BIG on the diagonal
    (gpsimd affine_select).
 2. Exact top-16 per row with the DVE max8/max_index/match_replace ISA:
    5 linear scans per 128-row tile.
 3. Neighbor gather with 16 indirect DMAs (one per neighbor rank; edges are
    ordered k-major so the offset columns are exactly the max_index outputs).
 4. Edge MLP with the first layer factored:
       relu(edge @ W1) = relu(x_i @ (W1a-W1c) + x_j @ (W1b+W1c))
    The x_i "broadcast over 16 neighbors" terms are injected via a second
    accumulating matmul against a constant 0/1 expansion matrix E
    (E[i, e] = 1 iff e//16 == i), so no elementwise broadcast is needed.
 5. Gate/aggregation algebra: max_k(y*gate) = gate*max_k(y) (gate>0), the
    x-channels of y are constant over k so their pooled value is just
    gate*x, and blast is folded in after the max-pool.
"""

import os
import sys

sys.path.insert(0, "/opt/trn_rl_repo")

import numpy as np

import concourse.bass as bass
import concourse.bacc as bacc
import concourse.tile as tile
from concourse import mybir
from concourse.bass_utils import run_bass_kernel_spmd

F32 = mybir.dt.float32
BF16 = mybir.dt.bfloat16
U32 = mybir.dt.uint32

B, N, D, G, K = 8, 4096, 64, 32, 16
COUT = D + 3 * G  # 160
NT = N // 128     # 32 row tiles
NEG = -3.0e38
AF = mybir.ActivationFunctionType
ALU = mybir.AluOpType


def build_nc(finalize: bool = True) -> bass.Bass:
    # Bacc (not plain Bass): its compile pass handles register allocation
    # and event-semaphore fusion that walrus codegen requires.
    nc = bacc.Bacc()

    # ---- DRAM parameters (per-core inputs) ----
    x_d = nc.dram_tensor("x", [N, D], BF16, kind="ExternalInput")
    R_d = nc.dram_tensor("Rm", [4, N], F32, kind="ExternalInput")    # [p^T; |p|^2]
    AR_d = nc.dram_tensor("AR", [D, 4 * D + G], BF16, kind="ExternalInput")  # [A | Wm_x]
    Bm_d = nc.dram_tensor("Bmat", [D, 4 * D], BF16, kind="ExternalInput")
    W2_d = nc.dram_tensor("W2", [4 * D, G], BF16, kind="ExternalInput")
    Wmh_d = nc.dram_tensor("Wmh", [G, G], BF16, kind="ExternalInput")
    Wg_d = nc.dram_tensor("Wg", [D + 2 * G, D + 2 * G], BF16, kind="ExternalInput")
    Wl_d = nc.dram_tensor("Wlast", [D + 2 * G, G], BF16, kind="ExternalInput")
    b1_d = nc.dram_tensor("b1", [128, 2], F32, kind="ExternalInput")
    b2_d = nc.dram_tensor("b2", [G, 1], F32, kind="ExternalInput")
    bmid_d = nc.dram_tensor("bmid", [G, 1], F32, kind="ExternalInput")
    bg_d = nc.dram_tensor("bg", [128, 1], F32, kind="ExternalInput")
    blast_d = nc.dram_tensor("blast", [G, 1], F32, kind="ExternalInput")
    # feature-major output; host transposes + upcasts
    out_d = nc.dram_tensor("out", [COUT, N], BF16, kind="ExternalOutput")

    E_COLS = 128 * K  # 2048 edges per row-tile
    NCH = 4           # edge chunks per row-tile
    CH = E_COLS // NCH  # 512

    with tile.TileContext(nc) as tc:
        with (
            tc.tile_pool(name="singles", bufs=1) as singles,
            tc.tile_pool(name="vals", bufs=2) as vals,
            tc.tile_pool(name="acts", bufs=2) as acts,
            tc.tile_pool(name="small", bufs=3) as small,
            tc.tile_pool(name="ps_val", bufs=2, space="PSUM") as ps_val,
            tc.tile_pool(name="ps_h1", bufs=2, space="PSUM") as ps_h1,
            tc.tile_pool(name="ps_a", bufs=2, space="PSUM") as ps_a,
            tc.tile_pool(name="ps_b", bufs=2, space="PSUM") as ps_b,
        ):
            # ---- load weights into SBUF once ----
            R_sb = singles.tile([4, N], F32)
            nc.sync.dma_start(out=R_sb, in_=R_d[:, :])
            AR_sb = singles.tile([D, 4 * D + G], BF16)
            nc.sync.dma_start(out=AR_sb, in_=AR_d[:, :])
            Bm_sb = singles.tile([D, 4 * D], BF16)
            nc.sync.dma_start(out=Bm_sb, in_=Bm_d[:, :])
            W2a_sb = singles.tile([128, G], BF16)
            nc.sync.dma_start(out=W2a_sb, in_=W2_d[0:128, :])
            W2b_sb = singles.tile([128, G], BF16)
            nc.sync.dma_start(out=W2b_sb, in_=W2_d[128:256, :])
            # Wmh sits at partition base 32 so its matmul rhs (yfm[32:64])
            # has a matching base partition.
            Wmh_sb = singles.tile([2 * G, G], BF16)
            nc.sync.dma_start(out=Wmh_sb[G:2 * G, :], in_=Wmh_d[:, :])
            Wg_sb = singles.tile([128, 128], BF16)
            nc.sync.dma_start(out=Wg_sb, in_=Wg_d[:, :])
            Wl_sb = singles.tile([128, G], BF16)
            nc.sync.dma_start(out=Wl_sb, in_=Wl_d[:, :])
            # rows 64:128 of Wlast again at base partition 0 (gxw matmul rhs)
            Wl2_sb = singles.tile([D, G], BF16)
            nc.sync.dma_start(out=Wl2_sb, in_=Wl_d[2 * G:128, :])
            b1_sb = singles.tile([128, 2], F32)
            nc.sync.dma_start(out=b1_sb, in_=b1_d[:, :])
            b2_sb = singles.tile([G, 1], F32)
            nc.sync.dma_start(out=b2_sb, in_=b2_d[:, :])
            bmid_sb = singles.tile([G, 1], F32)
            nc.sync.dma_start(out=bmid_sb, in_=bmid_d[:, :])
            bg_sb = singles.tile([128, 1], F32)
            nc.sync.dma_start(out=bg_sb, in_=bg_d[:, :])
            blast_sb = singles.tile([G, 1], F32)
            nc.sync.dma_start(out=blast_sb, in_=blast_d[:, :])

            # one-time gpsimd registers (to_reg per call exhausts the file)
            neg_reg = nc.gpsimd.to_reg(NEG)
            zero_reg = nc.gpsimd.to_reg(0.0)

            # ---- on-device constants (saves per-call upload) ----
            # bf16 identity: ones, then keep only the diagonal
            id_bf = singles.tile([128, 128], BF16)
            nc.vector.memset(id_bf, 1.0)
            nc.gpsimd.affine_select(
                out=id_bf, in_=id_bf, pattern=[[1, 128]],
                compare_op=ALU.is_equal, fill=zero_reg,
                base=0, channel_multiplier=-1)
            # E = identity tiled K times (k-major edge order:
            # E[i, 128*k + p] = (p == i))
            E_sb = singles.tile([128, E_COLS], BF16)
            for k in range(K):
                nc.scalar.copy(out=E_sb[:, 128 * k:128 * (k + 1)], in_=id_bf)
            # ranking lhs L = [2 p^T; -1] derived from R = [p^T; |p|^2] as
            # L = R*s1 + s2 with per-partition s1=[2,2,2,0], s2=[0,0,0,-1]
            # (engine ops must start at partition 0/32/64/96, so no direct
            # row-3 writes; affine_select picks out partition 3 instead)
            s1_sb = singles.tile([4, 1], F32)
            nc.vector.memset(s1_sb, 2.0)
            nc.gpsimd.affine_select(
                out=s1_sb, in_=s1_sb, pattern=[[1, 1]],
                compare_op=ALU.not_equal, fill=zero_reg,
                base=-3, channel_multiplier=1)
            s2_sb = singles.tile([4, 1], F32)
            nc.vector.memset(s2_sb, -1.0)
            nc.gpsimd.affine_select(
                out=s2_sb, in_=s2_sb, pattern=[[1, 1]],
                compare_op=ALU.is_equal, fill=zero_reg,
                base=-3, channel_multiplier=1)
            L_sb = singles.tile([4, N], F32)
            nc.vector.tensor_scalar(out=L_sb, in0=R_sb, scalar1=s1_sb,
                                    scalar2=s2_sb, op0=ALU.mult, op1=ALU.add)

            for t in range(NT):
                r0 = 128 * t

                # ---------- ranking matmul: val = L_t^T @ R ----------
                val_sb = vals.tile([128, N], F32, tag="val")
                for q in range(N // 512):
                    vps = ps_val.tile([128, 512], F32, tag="vps")
                    nc.tensor.matmul(vps, L_sb[:, r0:r0 + 128],
                                     R_sb[:, 512 * q:512 * (q + 1)],
                                     start=True, stop=True)
                    nc.scalar.copy(out=val_sb[:, 512 * q:512 * (q + 1)], in_=vps)

                # exclude self: val[r, r0+r] = -BIG (iota = j - p over the
                # diagonal 128-col block)
                nc.gpsimd.affine_select(
                    out=val_sb[:, r0:r0 + 128], in_=val_sb[:, r0:r0 + 128],
                    pattern=[[1, 128]], compare_op=ALU.not_equal, fill=neg_reg,
                    base=0, channel_multiplier=-1)

                # ---------- top-16 (max8 x2 rounds) ----------
                m1 = small.tile([128, 8], F32, tag="m1")
                i1 = small.tile([128, 8], U32, tag="i1")
                m2 = small.tile([128, 8], F32, tag="m2")
                i2 = small.tile([128, 8], U32, tag="i2")
                nc.vector.max(out=m1, in_=val_sb)
                nc.vector.max_index(out=i1, in_max=m1, in_values=val_sb)
                nc.vector.match_replace(out=val_sb, in_to_replace=m1,
                                        in_values=val_sb, imm_value=NEG)
                nc.vector.max(out=m2, in_=val_sb)
                nc.vector.max_index(out=i2, in_max=m2, in_values=val_sb)

                # ---------- gather neighbor features (HBM row gather) ----------
                # edges are k-major: block b holds the b-th nearest neighbor
                # of all 128 points, so the offsets are columns of i1/i2.
                # NOTE: one DMA per neighbor rank — batching all 16 into one
                # indirect DMA with a (128,16) offset tensor produces wrong
                # results on HW (walrus pairs offsets with dest rows in a
                # different order than the simulator).
                xg_sb = acts.tile([128, K, D], BF16, tag="xg")
                for b in range(K):
                    col = i1[:, b:b + 1] if b < 8 else i2[:, b - 8:b - 7]
                    nc.gpsimd.indirect_dma_start(
                        out=xg_sb[:, b, :], out_offset=None, in_=x_d[:, :],
                        in_offset=bass.IndirectOffsetOnAxis(ap=col, axis=0))

                # ---------- per-tile point-major x, P/R precompute ----------
                x_pm = small.tile([128, D], BF16, tag="x_pm")
                nc.sync.dma_start(out=x_pm, in_=x_d[r0:r0 + 128, :])
                xT_ps = ps_b.tile([D, 128], BF16, tag="psB")
                nc.tensor.transpose(xT_ps, x_pm, id_bf)
                xT_sb = small.tile([D, 128], F32, tag="xT")
                nc.scalar.copy(out=xT_sb, in_=xT_ps)
                xT_bf = small.tile([D, 128], BF16, tag="xTb")
                nc.scalar.copy(out=xT_bf, in_=xT_ps)

                PR_ps = ps_b.tile([128, 4 * D + G], F32, tag="psB")
                nc.tensor.matmul(PR_ps, xT_bf, AR_sb, start=True, stop=True)
                # bf16: lhsT of the E-expansion matmuls (pairs with bf16 E)
                PR_sb = small.tile([128, 4 * D + G], BF16, tag="PR")
                nc.scalar.copy(out=PR_sb, in_=PR_ps)

                # ---------- edge MLP ----------
                h1a = acts.tile([128, E_COLS], BF16, tag="h1a")
                h1b = acts.tile([128, E_COLS], BF16, tag="h1b")
                yfm = acts.tile([2 * G, E_COLS], BF16, tag="yfm")  # [m; h2]
                for c in range(NCH):
                    ec = slice(CH * c, CH * (c + 1))
                    # transpose gathered x into feature-major (64, 512)
                    xgT_ps = ps_b.tile([D, CH], BF16, tag="psB")
                    for bk in range(CH // 128):
                        nc.tensor.transpose(
                            xgT_ps[:, 128 * bk:128 * (bk + 1)],
                            xg_sb[:, (CH // 128) * c + bk, :], id_bf)
                    xgT = small.tile([D, CH], BF16, tag="xgT")
                    nc.scalar.copy(out=xgT, in_=xgT_ps)

                    # h1 = relu(Bm^T x_j + P_i + b1), two 128-ch halves
                    for h, h1_sb in ((0, h1a), (1, h1b)):
                        hps = ps_h1.tile([128, CH], F32, tag="h1ps")
                        nc.tensor.matmul(hps, Bm_sb[:, 128 * h:128 * (h + 1)],
                                         xgT, start=True, stop=False)
                        nc.tensor.matmul(hps, PR_sb[:, 128 * h:128 * (h + 1)],
                                         E_sb[:, ec], start=False, stop=True)
                        nc.scalar.activation(out=h1_sb[:, ec], in_=hps,
                                             func=AF.Relu,
                                             bias=b1_sb[:, h:h + 1])

                    # h2 = relu(W2^T h1 + b2) -> yfm rows 32:64
                    h2ps = ps_a.tile([G, CH], F32, tag="psA")
                    nc.tensor.matmul(h2ps, W2a_sb, h1a[:, ec], start=True, stop=False)
                    nc.tensor.matmul(h2ps, W2b_sb, h1b[:, ec], start=False, stop=True)
                    nc.scalar.activation(out=yfm[G:2 * G, ec], in_=h2ps,
                                         func=AF.Relu, bias=b2_sb)

                    # m = relu(Wmh^T h2 + R_i + bmid) -> yfm rows 0:32
                    mps = ps_a.tile([G, CH], F32, tag="psA")
                    nc.tensor.matmul(mps, Wmh_sb[G:2 * G, :], yfm[G:2 * G, ec],
                                     start=True, stop=False)
                    nc.tensor.matmul(mps, PR_sb[:, 4 * D:4 * D + G],
                                     E_sb[:, ec], start=False, stop=True)
                    nc.scalar.activation(out=yfm[0:G, ec], in_=mps,
                                         func=AF.Relu, bias=bmid_sb)

                # ---------- gate ----------
                # k-major edge order: position e = 128*k + point
                ymean_f = small.tile([2 * G, 128], F32, tag="ymean_f")
                nc.vector.tensor_reduce(
                    out=ymean_f,
                    in_=yfm.rearrange("p (k n) -> p n k", k=K),
                    axis=mybir.AxisListType.X, op=ALU.add)
                ymean = small.tile([128, 128], BF16, tag="ymean")
                nc.vector.tensor_copy(ymean[0:2 * G, :], ymean_f)
                nc.scalar.copy(out=ymean[2 * G:128, :], in_=xT_bf)

                gps = ps_b.tile([128, 128], F32, tag="psB")
                nc.tensor.matmul(gps, Wg_sb, ymean, start=True, stop=True)
                gate_fm = small.tile([128, 128], BF16, tag="gate_fm")
                nc.scalar.activation(out=gate_fm, in_=gps, func=AF.Sigmoid,
                                     bias=bg_sb)
                # gate rows 64:128 again at base partition 0, f32: the gx
                # multiply keeps the big x-channels at full sigmoid precision
                gate_hi = small.tile([D, 128], F32, tag="gate_hi")
                nc.scalar.activation(out=gate_hi, in_=gps[2 * G:128, :],
                                     func=AF.Sigmoid, bias=bg_sb[2 * G:128, :])
                gpm_ps = ps_b.tile([128, 128], BF16, tag="psB")
                nc.tensor.transpose(gpm_ps, gate_fm, id_bf)
                gate_pm = small.tile([128, 128], BF16, tag="gate_pm")
                nc.scalar.copy(out=gate_pm, in_=gpm_ps)

                # gx = gate[64:128] * x   (x-channels of y*gate, constant in k)
                gx_bf = small.tile([D, 128], BF16, tag="gx_bf")
                nc.vector.tensor_mul(gx_bf, gate_hi, xT_sb)
                gxw_ps = ps_b.tile([128, G], F32, tag="psB")
                nc.tensor.matmul(gxw_ps, gx_bf, Wl2_sb,
                                 start=True, stop=True)
                gxw_sb = small.tile([128, G], BF16, tag="gxw")
                nc.scalar.copy(out=gxw_sb, in_=gxw_ps)

                # ---------- gated last layer + max pooling ----------
                # each 512-edge chunk covers 4 neighbor ranks of all 128
                # points; keep a running max across chunks.
                zp_sb = small.tile([G, 128], F32, tag="zp")
                for c in range(NCH):
                    ec = slice(CH * c, CH * (c + 1))
                    ggps = ps_b.tile([2 * G, CH], F32, tag="psB")
                    nc.tensor.matmul(ggps, gate_pm[:, 0:2 * G], E_sb[:, ec],
                                     start=True, stop=True)
                    # yg = (gate broadcast) * yfm — ACT drains psum, the
                    # multiply runs on the otherwise-idle gpsimd (keeps the
                    # DVE free for the top-k scans)
                    gg_sb = small.tile([2 * G, CH], BF16, tag="gg")
                    nc.scalar.copy(out=gg_sb, in_=ggps)
                    yg_sb = small.tile([2 * G, CH], BF16, tag="yg")
                    nc.gpsimd.tensor_tensor(out=yg_sb, in0=gg_sb,
                                            in1=yfm[:, ec], op=ALU.mult)

                    zps = ps_a.tile([G, CH], F32, tag="psA")
                    nc.tensor.matmul(zps, Wl_sb[0:2 * G, :], yg_sb,
                                     start=True, stop=False)
                    nc.tensor.matmul(zps, gxw_sb, E_sb[:, ec],
                                     start=False, stop=True)
                    ztmp = small.tile([G, 128], F32, tag="ztmp")
                    nc.vector.tensor_reduce(
                        out=ztmp,
                        in_=zps.rearrange("p (k n) -> p n k", k=CH // 128),
                        axis=mybir.AxisListType.X, op=ALU.max)
                    if c == 0:
                        nc.vector.tensor_copy(zp_sb, ztmp)
                    else:
                        nc.vector.tensor_tensor(out=zp_sb, in0=zp_sb,
                                                in1=ztmp, op=ALU.max)

                ymax = small.tile([2 * G, 128], BF16, tag="ymax")
                nc.vector.tensor_reduce(
                    out=ymax, in_=yfm.rearrange("p (k n) -> p n k", k=K),
                    axis=mybir.AxisListType.X, op=ALU.max)

                # ---------- assemble output (feature-major, bf16) ----------
                zb_sb = small.tile([G, 128], BF16, tag="zb")
                nc.vector.tensor_add(zb_sb, zp_sb,
                                     blast_sb.to_broadcast([G, 128]))
                yout = small.tile([2 * G, 128], BF16, tag="yout")
                nc.vector.tensor_mul(yout, gate_fm[0:2 * G, :], ymax)

                nc.sync.dma_start(out=out_d[0:G, r0:r0 + 128], in_=zb_sb)
                nc.sync.dma_start(out=out_d[G:3 * G, r0:r0 + 128], in_=yout)
                nc.sync.dma_start(out=out_d[3 * G:COUT, r0:r0 + 128], in_=gx_bf)

    if finalize:
        nc.finalize()   # Bacc.compile: reg alloc, event sems, library loads
    return nc


_NC_CACHE = {}


def _get_nc():
    if "nc" not in _NC_CACHE:
        _NC_CACHE["nc"] = build_nc()
    return _NC_CACHE["nc"]


def _host_prep(inputs):
    """Shared (replicated) weight-derived arrays."""
    import ml_dtypes
    bf16 = ml_dtypes.bfloat16
    W1 = np.asarray(inputs["W1"], np.float32)
    Wmid = np.asarray(inputs["Wmid"], np.float32)
    A = W1[0:D] - W1[2 * D:3 * D]
    Bm = W1[D:2 * D] + W1[2 * D:3 * D]
    AR = np.concatenate([A, Wmid[G:G + D]], axis=1)          # (64, 288)
    Wg_adj = np.asarray(inputs["Wg"], np.float32).copy()
    Wg_adj[0:2 * G] /= K
    rep = {
        "AR": np.ascontiguousarray(AR).astype(bf16),
        "Bmat": np.ascontiguousarray(Bm).astype(bf16),
        "W2": np.asarray(inputs["W2"], np.float32).astype(bf16),
        "Wmh": np.ascontiguousarray(Wmid[0:G]).astype(bf16),
        "Wg": Wg_adj.astype(bf16),
        "Wlast": np.asarray(inputs["Wlast"], np.float32).astype(bf16),
        "b1": np.ascontiguousarray(
            np.asarray(inputs["b1"], np.float32).reshape(2, 128).T),
        "b2": np.asarray(inputs["b2"], np.float32).reshape(G, 1),
        "bmid": np.asarray(inputs["bmid"], np.float32).reshape(G, 1),
        "bg": np.asarray(inputs["bg"], np.float32).reshape(128, 1),
        "blast": np.asarray(inputs["blast"], np.float32).reshape(G, 1),
    }
    return rep


def make_in_maps(inputs):
    import ml_dtypes
    x = np.asarray(inputs["x"], np.float32)
    pos = np.asarray(inputs["pos"], np.float32)
    rep = _host_prep(inputs)
    in_maps = []
    for c in range(B):
        p = pos[c]
        sq = (p * p).sum(-1)
        R = np.concatenate([p.T, sq[None, :]], axis=0)
        m = dict(rep)
        m["x"] = np.ascontiguousarray(x[c]).astype(ml_dtypes.bfloat16)
        m["Rm"] = np.ascontiguousarray(R.astype(np.float32))
        in_maps.append(m)
    return in_maps


def kernel(**inputs) -> np.ndarray:
    nc = _get_nc()
    in_maps = make_in_maps(inputs)
    res = run_bass_kernel_spmd(nc, in_maps, list(range(B)))
    # out is feature-major bf16 [COUT, N]; transpose + upcast on host
    return np.stack([
        np.asarray(res.results[c]["out"]).T.astype(np.float32) for c in range(B)
    ])


if __name__ == "__main__":
    nc = build_nc()
    print("built ok:",
          sum(len(bb.instructions) for bb in nc.main_func.blocks), "instructions")


# revision 8
# speedup vs baseline: 2.6848x; 1.8487x over previous
"""DenseEdgeConv (gnn_message_passing) Trainium2 Bass kernel.

Problem: B=8 point clouds of N=4096 points. Per cloud: exact 16-NN by
Euclidean distance (excluding self), gather neighbor features, edge MLP,
channel gate, max-aggregation.  Output (B, N, 160) fp32.

Strategy: batch-parallel over 8 NeuronCores (1 cloud/core), no collectives.

The metric (wall time of a full dispatch) is transfer-dominated under the
axon PJRT tunnel, so the kernel minimizes per-call bytes:
 - x and all MLP weights ship as bf16 (the edge MLP already ran in bf16).
 - The 0/1 expansion matrix E, the transpose identity, and the ranking lhs
   L = [2p; -1] are generated on device instead of uploaded.
 - The output lands in DRAM as bf16 feature-major [160, N]; the host
   transposes and upcasts. This halves both the donated zero-output upload
   and the result download.
Ranking (distance matmul + top-k) stays fp32 end-to-end — neighbor
selection is the dominant error source and gets no dtype cut.

Per-core algorithm (all layouts "feature-major" = channels on partitions,
points/edges on the free axis, so matmuls chain on the PE without
transposes):

 1. Ranking matmul: val[i,j] = 2 p_i.p_j - |p_j|^2  (= -dist + const(i));
    self is always the row max, excluded by writing -BIG on the diagonal
    (gpsimd affine_select).
 2. Exact top-16 per row with the DVE max8/max_index/match_replace ISA:
    5 linear scans per 128-row tile.
 3. Neighbor gather with 16 indirect DMAs (one per neighbor rank; edges are
    ordered k-major so the offset columns are exactly the max_index outputs).
 4. Edge MLP with the first layer factored:
       relu(edge @ W1) = relu(x_i @ (W1a-W1c) + x_j @ (W1b+W1c))
    The x_i "broadcast over 16 neighbors" terms are injected via a second
    accumulating matmul against a constant 0/1 expansion matrix E
    (E[i, e] = 1 iff e//16 == i), so no elementwise broadcast is needed.
 5. Gate/aggregation algebra: max_k(y*gate) = gate*max_k(y) (gate>0), the
    x-channels of y are constant over k so their pooled value is just
    gate*x, and blast is folded in after the max-pool.
"""

import os
import sys

sys.path.insert(0, "/opt/trn_rl_repo")

import numpy as np

# Persistent XLA compilation cache: run_bass_kernel_spmd builds a fresh
# jit closure per call, which otherwise pays the full XLA+neuronx compile
# pipeline (~0.4s) on every invocation even with a warm NEFF cache.
os.environ.setdefault("JAX_COMPILATION_CACHE_DIR", "/tmp/jax_comp_cache")
import jax  # noqa: E402

jax.config.update("jax_compilation_cache_dir",
                  os.environ["JAX_COMPILATION_CACHE_DIR"])
jax.config.update("jax_persistent_cache_min_entry_size_bytes", -1)
jax.config.update("jax_persistent_cache_min_compile_time_secs", 0)

import concourse.bass as bass
import concourse.bacc as bacc
import concourse.tile as tile
from concourse import mybir
from concourse.bass_utils import run_bass_kernel_spmd

F32 = mybir.dt.float32
BF16 = mybir.dt.bfloat16
U32 = mybir.dt.uint32

B, N, D, G, K = 8, 4096, 64, 32, 16
COUT = D + 3 * G  # 160
NT = N // 128     # 32 row tiles
NEG = -3.0e38
AF = mybir.ActivationFunctionType
ALU = mybir.AluOpType


def build_nc(finalize: bool = True) -> bass.Bass:
    # Bacc (not plain Bass): its compile pass handles register allocation
    # and event-semaphore fusion that walrus codegen requires.
    nc = bacc.Bacc()

    # ---- DRAM parameters (per-core inputs) ----
    # Weights/biases travel as two flat dtype-packs: each extra PJRT input
    # buffer costs ~7ms of per-call dispatch under the axon tunnel.
    x_d = nc.dram_tensor("x", [N, D], BF16, kind="ExternalInput")
    pk16_d = nc.dram_tensor("pk16", [64512], BF16, kind="ExternalInput")
    pk32_d = nc.dram_tensor("pk32", [16864], F32, kind="ExternalInput")
    # feature-major output; host transposes + upcasts
    out_d = nc.dram_tensor("out", [COUT, N], BF16, kind="ExternalOutput")

    def pk16(off, p, f):
        return pk16_d[off:off + p * f].rearrange("(p f) -> p f", p=p)

    def pk32(off, p, f):
        return pk32_d[off:off + p * f].rearrange("(p f) -> p f", p=p)

    E_COLS = 128 * K  # 2048 edges per row-tile
    NCH = 4           # edge chunks per row-tile
    CH = E_COLS // NCH  # 512

    with tile.TileContext(nc) as tc:
        with (
            tc.tile_pool(name="singles", bufs=1) as singles,
            tc.tile_pool(name="vals", bufs=2) as vals,
            tc.tile_pool(name="acts", bufs=2) as acts,
            tc.tile_pool(name="small", bufs=3) as small,
            tc.tile_pool(name="ps_val", bufs=2, space="PSUM") as ps_val,
            tc.tile_pool(name="ps_h1", bufs=2, space="PSUM") as ps_h1,
            tc.tile_pool(name="ps_a", bufs=2, space="PSUM") as ps_a,
            tc.tile_pool(name="ps_b", bufs=2, space="PSUM") as ps_b,
        ):
            # ---- load weights into SBUF once (offsets match _host_prep) ----
            R_sb = singles.tile([4, N], F32)
            nc.sync.dma_start(out=R_sb, in_=pk32(0, 4, N))
            AR_sb = singles.tile([D, 4 * D + G], BF16)
            nc.sync.dma_start(out=AR_sb, in_=pk16(0, D, 4 * D + G))
            Bm_sb = singles.tile([D, 4 * D], BF16)
            nc.sync.dma_start(out=Bm_sb, in_=pk16(18432, D, 4 * D))
            W2a_sb = singles.tile([128, G], BF16)
            nc.sync.dma_start(out=W2a_sb, in_=pk16(34816, 128, G))
            W2b_sb = singles.tile([128, G], BF16)
            nc.sync.dma_start(out=W2b_sb, in_=pk16(38912, 128, G))
            # Wmh sits at partition base 32 so its matmul rhs (yfm[32:64])
            # has a matching base partition.
            Wmh_sb = singles.tile([2 * G, G], BF16)
            nc.sync.dma_start(out=Wmh_sb[G:2 * G, :], in_=pk16(43008, G, G))
            Wg_sb = singles.tile([128, 128], BF16)
            nc.sync.dma_start(out=Wg_sb, in_=pk16(44032, 128, 128))
            Wl_sb = singles.tile([128, G], BF16)
            nc.sync.dma_start(out=Wl_sb, in_=pk16(60416, 128, G))
            # rows 64:128 of Wlast again at base partition 0 (gxw matmul rhs)
            Wl2_sb = singles.tile([D, G], BF16)
            nc.sync.dma_start(out=Wl2_sb, in_=pk16(62464, D, G))
            b1_sb = singles.tile([128, 2], F32)
            nc.sync.dma_start(out=b1_sb, in_=pk32(16384, 128, 2))
            b2_sb = singles.tile([G, 1], F32)
            nc.sync.dma_start(out=b2_sb, in_=pk32(16640, G, 1))
            bmid_sb = singles.tile([G, 1], F32)
            nc.sync.dma_start(out=bmid_sb, in_=pk32(16672, G, 1))
            bg_sb = singles.tile([128, 1], F32)
            nc.sync.dma_start(out=bg_sb, in_=pk32(16704, 128, 1))
            blast_sb = singles.tile([G, 1], F32)
            nc.sync.dma_start(out=blast_sb, in_=pk32(16832, G, 1))

            # one-time gpsimd registers (to_reg per call exhausts the file)
            neg_reg = nc.gpsimd.to_reg(NEG)
            zero_reg = nc.gpsimd.to_reg(0.0)

            # ---- on-device constants (saves per-call upload) ----
            # bf16 identity: ones, then keep only the diagonal
            id_bf = singles.tile([128, 128], BF16)
            nc.vector.memset(id_bf, 1.0)
            nc.gpsimd.affine_select(
                out=id_bf, in_=id_bf, pattern=[[1, 128]],
                compare_op=ALU.is_equal, fill=zero_reg,
                base=0, channel_multiplier=-1)
            # E = identity tiled K times (k-major edge order:
            # E[i, 128*k + p] = (p == i))
            E_sb = singles.tile([128, E_COLS], BF16)
            for k in range(K):
                nc.scalar.copy(out=E_sb[:, 128 * k:128 * (k + 1)], in_=id_bf)
            # ranking lhs L = [2 p^T; -1] derived from R = [p^T; |p|^2] as
            # L = R*s1 + s2 with per-partition s1=[2,2,2,0], s2=[0,0,0,-1]
            # (engine ops must start at partition 0/32/64/96, so no direct
            # row-3 writes; affine_select picks out partition 3 instead)
            s1_sb = singles.tile([4, 1], F32)
            nc.vector.memset(s1_sb, 2.0)
            nc.gpsimd.affine_select(
                out=s1_sb, in_=s1_sb, pattern=[[1, 1]],
                compare_op=ALU.not_equal, fill=zero_reg,
                base=-3, channel_multiplier=1)
            s2_sb = singles.tile([4, 1], F32)
            nc.vector.memset(s2_sb, -1.0)
            nc.gpsimd.affine_select(
                out=s2_sb, in_=s2_sb, pattern=[[1, 1]],
                compare_op=ALU.is_equal, fill=zero_reg,
                base=-3, channel_multiplier=1)
            L_sb = singles.tile([4, N], F32)
            nc.vector.tensor_scalar(out=L_sb, in0=R_sb, scalar1=s1_sb,
                                    scalar2=s2_sb, op0=ALU.mult, op1=ALU.add)

            for t in range(NT):
                r0 = 128 * t

                # ---------- ranking matmul: val = L_t^T @ R ----------
                val_sb = vals.tile([128, N], F32, tag="val")
                for q in range(N // 512):
                    vps = ps_val.tile([128, 512], F32, tag="vps")
                    nc.tensor.matmul(vps, L_sb[:, r0:r0 + 128],
                                     R_sb[:, 512 * q:512 * (q + 1)],
                                     start=True, stop=True)
                    nc.scalar.copy(out=val_sb[:, 512 * q:512 * (q + 1)], in_=vps)

                # exclude self: val[r, r0+r] = -BIG (iota = j - p over the
                # diagonal 128-col block)
                nc.gpsimd.affine_select(
                    out=val_sb[:, r0:r0 + 128], in_=val_sb[:, r0:r0 + 128],
                    pattern=[[1, 128]], compare_op=ALU.not_equal, fill=neg_reg,
                    base=0, channel_multiplier=-1)

                # ---------- top-16 (max8 x2 rounds) ----------
                m1 = small.tile([128, 8], F32, tag="m1")
                i1 = small.tile([128, 8], U32, tag="i1")
                m2 = small.tile([128, 8], F32, tag="m2")
                i2 = small.tile([128, 8], U32, tag="i2")
                nc.vector.max(out=m1, in_=val_sb)
                nc.vector.max_index(out=i1, in_max=m1, in_values=val_sb)
                nc.vector.match_replace(out=val_sb, in_to_replace=m1,
                                        in_values=val_sb, imm_value=NEG)
                nc.vector.max(out=m2, in_=val_sb)
                nc.vector.max_index(out=i2, in_max=m2, in_values=val_sb)

                # ---------- gather neighbor features (HBM row gather) ----------
                # edges are k-major: block b holds the b-th nearest neighbor
                # of all 128 points, so the offsets are columns of i1/i2.
                # NOTE: one DMA per neighbor rank — batching all 16 into one
                # indirect DMA with a (128,16) offset tensor produces wrong
                # results on HW (walrus pairs offsets with dest rows in a
                # different order than the simulator).
                xg_sb = acts.tile([128, K, D], BF16, tag="xg")
                for b in range(K):
                    col = i1[:, b:b + 1] if b < 8 else i2[:, b - 8:b - 7]
                    nc.gpsimd.indirect_dma_start(
                        out=xg_sb[:, b, :], out_offset=None, in_=x_d[:, :],
                        in_offset=bass.IndirectOffsetOnAxis(ap=col, axis=0))

                # ---------- per-tile point-major x, P/R precompute ----------
                x_pm = small.tile([128, D], BF16, tag="x_pm")
                nc.sync.dma_start(out=x_pm, in_=x_d[r0:r0 + 128, :])
                xT_ps = ps_b.tile([D, 128], BF16, tag="psB")
                nc.tensor.transpose(xT_ps, x_pm, id_bf)
                xT_sb = small.tile([D, 128], F32, tag="xT")
                nc.scalar.copy(out=xT_sb, in_=xT_ps)
                xT_bf = small.tile([D, 128], BF16, tag="xTb")
                nc.scalar.copy(out=xT_bf, in_=xT_ps)

                PR_ps = ps_b.tile([128, 4 * D + G], F32, tag="psB")
                nc.tensor.matmul(PR_ps, xT_bf, AR_sb, start=True, stop=True)
                # bf16: lhsT of the E-expansion matmuls (pairs with bf16 E)
                PR_sb = small.tile([128, 4 * D + G], BF16, tag="PR")
                nc.scalar.copy(out=PR_sb, in_=PR_ps)

                # ---------- edge MLP ----------
                h1a = acts.tile([128, E_COLS], BF16, tag="h1a")
                h1b = acts.tile([128, E_COLS], BF16, tag="h1b")
                yfm = acts.tile([2 * G, E_COLS], BF16, tag="yfm")  # [m; h2]
                for c in range(NCH):
                    ec = slice(CH * c, CH * (c + 1))
                    # transpose gathered x into feature-major (64, 512)
                    xgT_ps = ps_b.tile([D, CH], BF16, tag="psB")
                    for bk in range(CH // 128):
                        nc.tensor.transpose(
                            xgT_ps[:, 128 * bk:128 * (bk + 1)],
                            xg_sb[:, (CH // 128) * c + bk, :], id_bf)
                    xgT = small.tile([D, CH], BF16, tag="xgT")
                    nc.scalar.copy(out=xgT, in_=xgT_ps)

                    # h1 = relu(Bm^T x_j + P_i + b1), two 128-ch halves
                    for h, h1_sb in ((0, h1a), (1, h1b)):
                        hps = ps_h1.tile([128, CH], F32, tag="h1ps")
                        nc.tensor.matmul(hps, Bm_sb[:, 128 * h:128 * (h + 1)],
                                         xgT, start=True, stop=False)
                        nc.tensor.matmul(hps, PR_sb[:, 128 * h:128 * (h + 1)],
                                         E_sb[:, ec], start=False, stop=True)
                        nc.scalar.activation(out=h1_sb[:, ec], in_=hps,
                                             func=AF.Relu,
                                             bias=b1_sb[:, h:h + 1])

                    # h2 = relu(W2^T h1 + b2) -> yfm rows 32:64
                    h2ps = ps_a.tile([G, CH], F32, tag="psA")
                    nc.tensor.matmul(h2ps, W2a_sb, h1a[:, ec], start=True, stop=False)
                    nc.tensor.matmul(h2ps, W2b_sb, h1b[:, ec], start=False, stop=True)
                    nc.scalar.activation(out=yfm[G:2 * G, ec], in_=h2ps,
                                         func=AF.Relu, bias=b2_sb)

                    # m = relu(Wmh^T h2 + R_i + bmid) -> yfm rows 0:32
                    mps = ps_a.tile([G, CH], F32, tag="psA")
                    nc.tensor.matmul(mps, Wmh_sb[G:2 * G, :], yfm[G:2 * G, ec],
                                     start=True, stop=False)
                    nc.tensor.matmul(mps, PR_sb[:, 4 * D:4 * D + G],
                                     E_sb[:, ec], start=False, stop=True)
                    nc.scalar.activation(out=yfm[0:G, ec], in_=mps,
                                         func=AF.Relu, bias=bmid_sb)

                # ---------- gate ----------
                # k-major edge order: position e = 128*k + point
                ymean_f = small.tile([2 * G, 128], F32, tag="ymean_f")
                nc.vector.tensor_reduce(
                    out=ymean_f,
                    in_=yfm.rearrange("p (k n) -> p n k", k=K),
                    axis=mybir.AxisListType.X, op=ALU.add)
                ymean = small.tile([128, 128], BF16, tag="ymean")
                nc.vector.tensor_copy(ymean[0:2 * G, :], ymean_f)
                nc.scalar.copy(out=ymean[2 * G:128, :], in_=xT_bf)

                gps = ps_b.tile([128, 128], F32, tag="psB")
                nc.tensor.matmul(gps, Wg_sb, ymean, start=True, stop=True)
                gate_fm = small.tile([128, 128], BF16, tag="gate_fm")
                nc.scalar.activation(out=gate_fm, in_=gps, func=AF.Sigmoid,
                                     bias=bg_sb)
                # gate rows 64:128 again at base partition 0, f32: the gx
                # multiply keeps the big x-channels at full sigmoid precision
                gate_hi = small.tile([D, 128], F32, tag="gate_hi")
                nc.scalar.activation(out=gate_hi, in_=gps[2 * G:128, :],
                                     func=AF.Sigmoid, bias=bg_sb[2 * G:128, :])
                gpm_ps = ps_b.tile([128, 128], BF16, tag="psB")
                nc.tensor.transpose(gpm_ps, gate_fm, id_bf)
                gate_pm = small.tile([128, 128], BF16, tag="gate_pm")
                nc.scalar.copy(out=gate_pm, in_=gpm_ps)

                # gx = gate[64:128] * x   (x-channels of y*gate, constant in k)
                gx_bf = small.tile([D, 128], BF16, tag="gx_bf")
                nc.vector.tensor_mul(gx_bf, gate_hi, xT_sb)
                gxw_ps = ps_b.tile([128, G], F32, tag="psB")
                nc.tensor.matmul(gxw_ps, gx_bf, Wl2_sb,
                                 start=True, stop=True)
                gxw_sb = small.tile([128, G], BF16, tag="gxw")
                nc.scalar.copy(out=gxw_sb, in_=gxw_ps)

                # ---------- gated last layer + max pooling ----------
                # each 512-edge chunk covers 4 neighbor ranks of all 128
                # points; keep a running max across chunks.
                zp_sb = small.tile([G, 128], F32, tag="zp")
                for c in range(NCH):
                    ec = slice(CH * c, CH * (c + 1))
                    ggps = ps_b.tile([2 * G, CH], F32, tag="psB")
                    nc.tensor.matmul(ggps, gate_pm[:, 0:2 * G], E_sb[:, ec],
                                     start=True, stop=True)
                    # yg = (gate broadcast) * yfm — ACT drains psum, the
                    # multiply runs on the otherwise-idle gpsimd (keeps the
                    # DVE free for the top-k scans)
                    gg_sb = small.tile([2 * G, CH], BF16, tag="gg")
                    nc.scalar.copy(out=gg_sb, in_=ggps)
                    yg_sb = small.tile([2 * G, CH], BF16, tag="yg")
                    nc.gpsimd.tensor_tensor(out=yg_sb, in0=gg_sb,
                                            in1=yfm[:, ec], op=ALU.mult)

                    zps = ps_a.tile([G, CH], F32, tag="psA")
                    nc.tensor.matmul(zps, Wl_sb[0:2 * G, :], yg_sb,
                                     start=True, stop=False)
                    nc.tensor.matmul(zps, gxw_sb, E_sb[:, ec],
                                     start=False, stop=True)
                    ztmp = small.tile([G, 128], F32, tag="ztmp")
                    nc.vector.tensor_reduce(
                        out=ztmp,
                        in_=zps.rearrange("p (k n) -> p n k", k=CH // 128),
                        axis=mybir.AxisListType.X, op=ALU.max)
                    if c == 0:
                        nc.vector.tensor_copy(zp_sb, ztmp)
                    else:
                        nc.vector.tensor_tensor(out=zp_sb, in0=zp_sb,
                                                in1=ztmp, op=ALU.max)

                ymax = small.tile([2 * G, 128], BF16, tag="ymax")
                nc.vector.tensor_reduce(
                    out=ymax, in_=yfm.rearrange("p (k n) -> p n k", k=K),
                    axis=mybir.AxisListType.X, op=ALU.max)

                # ---------- assemble output (feature-major, bf16) ----------
                zb_sb = small.tile([G, 128], BF16, tag="zb")
                nc.vector.tensor_add(zb_sb, zp_sb,
                                     blast_sb.to_broadcast([G, 128]))
                yout = small.tile([2 * G, 128], BF16, tag="yout")
                nc.vector.tensor_mul(yout, gate_fm[0:2 * G, :], ymax)

                nc.sync.dma_start(out=out_d[0:G, r0:r0 + 128], in_=zb_sb)
                nc.sync.dma_start(out=out_d[G:3 * G, r0:r0 + 128], in_=yout)
                nc.sync.dma_start(out=out_d[3 * G:COUT, r0:r0 + 128], in_=gx_bf)

    if finalize:
        nc.finalize()   # Bacc.compile: reg alloc, event sems, library loads
    return nc


_NC_CACHE = {}


def _get_nc():
    if "nc" not in _NC_CACHE:
        _NC_CACHE["nc"] = build_nc()
    return _NC_CACHE["nc"]


def _host_prep(inputs):
    """Shared (replicated) weight-derived arrays, packed per dtype.

    pk16 element offsets: AR@0 (64x288), Bm@18432 (64x256), W2@34816
    (256x32), Wmh@43008 (32x32), Wg@44032 (128x128), Wl@60416 (128x32).
    pk32: Rm is per-core and prepended in make_in_maps; biases follow at
    16384: b1 (128x2), b2@16640, bmid@16672, bg@16704, blast@16832.
    """
    import ml_dtypes
    bf16 = ml_dtypes.bfloat16
    W1 = np.asarray(inputs["W1"], np.float32)
    Wmid = np.asarray(inputs["Wmid"], np.float32)
    A = W1[0:D] - W1[2 * D:3 * D]
    Bm = W1[D:2 * D] + W1[2 * D:3 * D]
    AR = np.concatenate([A, Wmid[G:G + D]], axis=1)          # (64, 288)
    Wg_adj = np.asarray(inputs["Wg"], np.float32).copy()
    Wg_adj[0:2 * G] /= K
    pk16 = np.concatenate([
        np.ascontiguousarray(AR).astype(bf16).ravel(),
        np.ascontiguousarray(Bm).astype(bf16).ravel(),
        np.asarray(inputs["W2"], np.float32).astype(bf16).ravel(),
        np.ascontiguousarray(Wmid[0:G]).astype(bf16).ravel(),
        Wg_adj.astype(bf16).ravel(),
        np.asarray(inputs["Wlast"], np.float32).astype(bf16).ravel(),
    ])
    assert pk16.size == 64512
    bias32 = np.concatenate([
        np.ascontiguousarray(
            np.asarray(inputs["b1"], np.float32).reshape(2, 128).T).ravel(),
        np.asarray(inputs["b2"], np.float32).ravel(),
        np.asarray(inputs["bmid"], np.float32).ravel(),
        np.asarray(inputs["bg"], np.float32).ravel(),
        np.asarray(inputs["blast"], np.float32).ravel(),
    ]).astype(np.float32)
    return pk16, bias32


def make_in_maps(inputs):
    import ml_dtypes
    x = np.asarray(inputs["x"], np.float32)
    pos = np.asarray(inputs["pos"], np.float32)
    pk16, bias32 = _host_prep(inputs)
    in_maps = []
    for c in range(B):
        p = pos[c]
        sq = (p * p).sum(-1)
        R = np.concatenate([p.T, sq[None, :]], axis=0).astype(np.float32)
        in_maps.append({
            "x": np.ascontiguousarray(x[c]).astype(ml_dtypes.bfloat16),
            "pk16": pk16,
            "pk32": np.concatenate([R.ravel(), bias32]),
        })
    return in_maps


def kernel(**inputs) -> np.ndarray:
    nc = _get_nc()
    in_maps = make_in_maps(inputs)
    res = run_bass_kernel_spmd(nc, in_maps, list(range(B)))
    # out is feature-major bf16 [COUT, N]; transpose + upcast on host
    return np.stack([
        np.asarray(res.results[c]["out"]).T.astype(np.float32) for c in range(B)
    ])


if __name__ == "__main__":
    nc = build_nc()
    print("built ok:",
          sum(len(bb.instructions) for bb in nc.main_func.blocks), "instructions")


# revision 9
# speedup vs baseline: 3.2613x; 1.2147x over previous
"""DenseEdgeConv (gnn_message_passing) Trainium2 Bass kernel.

Problem: B=8 point clouds of N=4096 points. Per cloud: exact 16-NN by
Euclidean distance (excluding self), gather neighbor features, edge MLP,
channel gate, max-aggregation.  Output (B, N, 160) fp32.

Strategy: batch-parallel over 8 NeuronCores (1 cloud/core), no collectives.

The metric (wall time of a full dispatch) is transfer-dominated under the
axon PJRT tunnel, so the kernel minimizes per-call bytes:
 - x and all MLP weights ship as bf16 (the edge MLP already ran in bf16).
 - The 0/1 expansion matrix E, the transpose identity, and the ranking lhs
   L = [2p; -1] are generated on device instead of uploaded.
 - The output lands in DRAM as bf16 feature-major [160, N]; the host
   transposes and upcasts. This halves both the donated zero-output upload
   and the result download.
Ranking (distance matmul + top-k) stays fp32 end-to-end — neighbor
selection is the dominant error source and gets no dtype cut.

Per-core algorithm (all layouts "feature-major" = channels on partitions,
points/edges on the free axis, so matmuls chain on the PE without
transposes):

 1. Ranking matmul: val[i,j] = 2 p_i.p_j - |p_j|^2  (= -dist + const(i));
    self is always the row max, excluded by writing -BIG on the diagonal
    (gpsimd affine_select).
 2. Exact top-16 per row with the DVE max8/max_index/match_replace ISA:
    5 linear scans per 128-row tile.
 3. Neighbor gather with 16 indirect DMAs (one per neighbor rank; edges are
    ordered k-major so the offset columns are exactly the max_index outputs).
 4. Edge MLP with the first layer factored:
       relu(edge @ W1) = relu(x_i @ (W1a-W1c) + x_j @ (W1b+W1c))
    The x_i "broadcast over 16 neighbors" terms are injected via a second
    accumulating matmul against a constant 0/1 expansion matrix E
    (E[i, e] = 1 iff e//16 == i), so no elementwise broadcast is needed.
 5. Gate/aggregation algebra: max_k(y*gate) = gate*max_k(y) (gate>0), the
    x-channels of y are constant over k so their pooled value is just
    gate*x, and blast is folded in after the max-pool.
"""

import os
import sys

sys.path.insert(0, "/opt/trn_rl_repo")

import numpy as np

# Persistent XLA compilation cache: run_bass_kernel_spmd builds a fresh
# jit closure per call, which otherwise pays the full XLA+neuronx compile
# pipeline (~0.4s) on every invocation even with a warm NEFF cache.
os.environ.setdefault("JAX_COMPILATION_CACHE_DIR", "/tmp/jax_comp_cache")
import jax  # noqa: E402

jax.config.update("jax_compilation_cache_dir",
                  os.environ["JAX_COMPILATION_CACHE_DIR"])
jax.config.update("jax_persistent_cache_min_entry_size_bytes", -1)
jax.config.update("jax_persistent_cache_min_compile_time_secs", 0)

import concourse.bass as bass
import concourse.bacc as bacc
import concourse.tile as tile
from concourse import mybir
from concourse.bass_utils import run_bass_kernel_spmd

F32 = mybir.dt.float32
BF16 = mybir.dt.bfloat16
U32 = mybir.dt.uint32

B, N, D, G, K = 8, 4096, 64, 32, 16
COUT = D + 3 * G  # 160
NT = N // 128     # 32 row tiles
NEG = -3.0e38
AF = mybir.ActivationFunctionType
ALU = mybir.AluOpType


def build_nc(finalize: bool = True) -> bass.Bass:
    # Bacc (not plain Bass): its compile pass handles register allocation
    # and event-semaphore fusion that walrus codegen requires.
    nc = bacc.Bacc()

    # ---- DRAM parameters (per-core inputs) ----
    # Weights/biases travel as two flat dtype-packs: each extra PJRT input
    # buffer costs ~7ms of per-call dispatch under the axon tunnel.
    x_d = nc.dram_tensor("x", [N, D], BF16, kind="ExternalInput")
    pk16_d = nc.dram_tensor("pk16", [64512], BF16, kind="ExternalInput")
    pk32_d = nc.dram_tensor("pk32", [16864], F32, kind="ExternalInput")
    # feature-major output; host transposes + upcasts
    out_d = nc.dram_tensor("out", [COUT, N], BF16, kind="ExternalOutput")

    def pk16(off, p, f):
        return pk16_d[off:off + p * f].rearrange("(p f) -> p f", p=p)

    def pk32(off, p, f):
        return pk32_d[off:off + p * f].rearrange("(p f) -> p f", p=p)

    E_COLS = 128 * K  # 2048 edges per row-tile
    NCH = 4           # edge chunks per row-tile
    CH = E_COLS // NCH  # 512

    with tile.TileContext(nc) as tc:
        with (
            tc.tile_pool(name="singles", bufs=1) as singles,
            tc.tile_pool(name="vals", bufs=2) as vals,
            tc.tile_pool(name="acts", bufs=2) as acts,
            tc.tile_pool(name="small", bufs=3) as small,
            tc.tile_pool(name="ps_val", bufs=2, space="PSUM") as ps_val,
            tc.tile_pool(name="ps_h1", bufs=2, space="PSUM") as ps_h1,
            tc.tile_pool(name="ps_a", bufs=2, space="PSUM") as ps_a,
            tc.tile_pool(name="ps_b", bufs=2, space="PSUM") as ps_b,
        ):
            # ---- load weights into SBUF once (offsets match _host_prep) ----
            R_sb = singles.tile([4, N], F32)
            nc.sync.dma_start(out=R_sb, in_=pk32(0, 4, N))
            AR_sb = singles.tile([D, 4 * D + G], BF16)
            nc.sync.dma_start(out=AR_sb, in_=pk16(0, D, 4 * D + G))
            Bm_sb = singles.tile([D, 4 * D], BF16)
            nc.sync.dma_start(out=Bm_sb, in_=pk16(18432, D, 4 * D))
            W2a_sb = singles.tile([128, G], BF16)
            nc.sync.dma_start(out=W2a_sb, in_=pk16(34816, 128, G))
            W2b_sb = singles.tile([128, G], BF16)
            nc.sync.dma_start(out=W2b_sb, in_=pk16(38912, 128, G))
            # Wmh sits at partition base 32 so its matmul rhs (yfm[32:64])
            # has a matching base partition.
            Wmh_sb = singles.tile([2 * G, G], BF16)
            nc.sync.dma_start(out=Wmh_sb[G:2 * G, :], in_=pk16(43008, G, G))
            Wg_sb = singles.tile([128, 128], BF16)
            nc.sync.dma_start(out=Wg_sb, in_=pk16(44032, 128, 128))
            Wl_sb = singles.tile([128, G], BF16)
            nc.sync.dma_start(out=Wl_sb, in_=pk16(60416, 128, G))
            # rows 64:128 of Wlast again at base partition 0 (gxw matmul rhs)
            Wl2_sb = singles.tile([D, G], BF16)
            nc.sync.dma_start(out=Wl2_sb, in_=pk16(62464, D, G))
            b1_sb = singles.tile([128, 2], F32)
            nc.sync.dma_start(out=b1_sb, in_=pk32(16384, 128, 2))
            b2_sb = singles.tile([G, 1], F32)
            nc.sync.dma_start(out=b2_sb, in_=pk32(16640, G, 1))
            bmid_sb = singles.tile([G, 1], F32)
            nc.sync.dma_start(out=bmid_sb, in_=pk32(16672, G, 1))
            bg_sb = singles.tile([128, 1], F32)
            nc.sync.dma_start(out=bg_sb, in_=pk32(16704, 128, 1))
            blast_sb = singles.tile([G, 1], F32)
            nc.sync.dma_start(out=blast_sb, in_=pk32(16832, G, 1))

            # one-time gpsimd registers (to_reg per call exhausts the file)
            neg_reg = nc.gpsimd.to_reg(NEG)
            zero_reg = nc.gpsimd.to_reg(0.0)

            # ---- on-device constants (saves per-call upload) ----
            # bf16 identity: ones, then keep only the diagonal
            id_bf = singles.tile([128, 128], BF16)
            nc.vector.memset(id_bf, 1.0)
            nc.gpsimd.affine_select(
                out=id_bf, in_=id_bf, pattern=[[1, 128]],
                compare_op=ALU.is_equal, fill=zero_reg,
                base=0, channel_multiplier=-1)
            # E = identity tiled K times (k-major edge order:
            # E[i, 128*k + p] = (p == i))
            E_sb = singles.tile([128, E_COLS], BF16)
            for k in range(K):
                nc.scalar.copy(out=E_sb[:, 128 * k:128 * (k + 1)], in_=id_bf)
            # ranking lhs L = [2 p^T; -1] derived from R = [p^T; |p|^2] as
            # L = R*s1 + s2 with per-partition s1=[2,2,2,0], s2=[0,0,0,-1]
            # (engine ops must start at partition 0/32/64/96, so no direct
            # row-3 writes; affine_select picks out partition 3 instead)
            s1_sb = singles.tile([4, 1], F32)
            nc.vector.memset(s1_sb, 2.0)
            nc.gpsimd.affine_select(
                out=s1_sb, in_=s1_sb, pattern=[[1, 1]],
                compare_op=ALU.not_equal, fill=zero_reg,
                base=-3, channel_multiplier=1)
            s2_sb = singles.tile([4, 1], F32)
            nc.vector.memset(s2_sb, -1.0)
            nc.gpsimd.affine_select(
                out=s2_sb, in_=s2_sb, pattern=[[1, 1]],
                compare_op=ALU.is_equal, fill=zero_reg,
                base=-3, channel_multiplier=1)
            L_sb = singles.tile([4, N], F32)
            nc.vector.tensor_scalar(out=L_sb, in0=R_sb, scalar1=s1_sb,
                                    scalar2=s2_sb, op0=ALU.mult, op1=ALU.add)

            for t in range(NT):
                r0 = 128 * t

                # ---------- ranking matmul: val = L_t^T @ R ----------
                val_sb = vals.tile([128, N], F32, tag="val")
                for q in range(N // 512):
                    vps = ps_val.tile([128, 512], F32, tag="vps")
                    nc.tensor.matmul(vps, L_sb[:, r0:r0 + 128],
                                     R_sb[:, 512 * q:512 * (q + 1)],
                                     start=True, stop=True)
                    nc.scalar.copy(out=val_sb[:, 512 * q:512 * (q + 1)], in_=vps)

                # exclude self: val[r, r0+r] = -BIG (iota = j - p over the
                # diagonal 128-col block)
                nc.gpsimd.affine_select(
                    out=val_sb[:, r0:r0 + 128], in_=val_sb[:, r0:r0 + 128],
                    pattern=[[1, 128]], compare_op=ALU.not_equal, fill=neg_reg,
                    base=0, channel_multiplier=-1)

                # ---------- top-16 (max8 x2 rounds) ----------
                m1 = small.tile([128, 8], F32, tag="m1")
                i1 = small.tile([128, 8], U32, tag="i1")
                m2 = small.tile([128, 8], F32, tag="m2")
                i2 = small.tile([128, 8], U32, tag="i2")
                nc.vector.max(out=m1, in_=val_sb)
                nc.vector.max_index(out=i1, in_max=m1, in_values=val_sb)
                nc.vector.match_replace(out=val_sb, in_to_replace=m1,
                                        in_values=val_sb, imm_value=NEG)
                nc.vector.max(out=m2, in_=val_sb)
                nc.vector.max_index(out=i2, in_max=m2, in_values=val_sb)

                # ---------- gather neighbor features (HBM row gather) ----------
                # edges are k-major: block b holds the b-th nearest neighbor
                # of all 128 points, so the offsets are columns of i1/i2.
                # NOTE: one DMA per neighbor rank — batching all 16 into one
                # indirect DMA with a (128,16) offset tensor produces wrong
                # results on HW (walrus pairs offsets with dest rows in a
                # different order than the simulator).
                xg_sb = acts.tile([128, K, D], BF16, tag="xg")
                for b in range(K):
                    col = i1[:, b:b + 1] if b < 8 else i2[:, b - 8:b - 7]
                    nc.gpsimd.indirect_dma_start(
                        out=xg_sb[:, b, :], out_offset=None, in_=x_d[:, :],
                        in_offset=bass.IndirectOffsetOnAxis(ap=col, axis=0))

                # ---------- per-tile point-major x, P/R precompute ----------
                x_pm = small.tile([128, D], BF16, tag="x_pm")
                nc.sync.dma_start(out=x_pm, in_=x_d[r0:r0 + 128, :])
                xT_ps = ps_b.tile([D, 128], BF16, tag="psB")
                nc.tensor.transpose(xT_ps, x_pm, id_bf)
                xT_sb = small.tile([D, 128], F32, tag="xT")
                nc.scalar.copy(out=xT_sb, in_=xT_ps)
                xT_bf = small.tile([D, 128], BF16, tag="xTb")
                nc.scalar.copy(out=xT_bf, in_=xT_ps)

                PR_ps = ps_b.tile([128, 4 * D + G], F32, tag="psB")
                nc.tensor.matmul(PR_ps, xT_bf, AR_sb, start=True, stop=True)
                # bf16: lhsT of the E-expansion matmuls (pairs with bf16 E)
                PR_sb = small.tile([128, 4 * D + G], BF16, tag="PR")
                nc.scalar.copy(out=PR_sb, in_=PR_ps)

                # ---------- edge MLP ----------
                h1a = acts.tile([128, E_COLS], BF16, tag="h1a")
                h1b = acts.tile([128, E_COLS], BF16, tag="h1b")
                yfm = acts.tile([2 * G, E_COLS], BF16, tag="yfm")  # [m; h2]
                for c in range(NCH):
                    ec = slice(CH * c, CH * (c + 1))
                    # transpose gathered x into feature-major (64, 512)
                    xgT_ps = ps_b.tile([D, CH], BF16, tag="psB")
                    for bk in range(CH // 128):
                        nc.tensor.transpose(
                            xgT_ps[:, 128 * bk:128 * (bk + 1)],
                            xg_sb[:, (CH // 128) * c + bk, :], id_bf)
                    xgT = small.tile([D, CH], BF16, tag="xgT")
                    nc.scalar.copy(out=xgT, in_=xgT_ps)

                    # h1 = relu(Bm^T x_j + P_i + b1), two 128-ch halves
                    for h, h1_sb in ((0, h1a), (1, h1b)):
                        hps = ps_h1.tile([128, CH], F32, tag="h1ps")
                        nc.tensor.matmul(hps, Bm_sb[:, 128 * h:128 * (h + 1)],
                                         xgT, start=True, stop=False)
                        nc.tensor.matmul(hps, PR_sb[:, 128 * h:128 * (h + 1)],
                                         E_sb[:, ec], start=False, stop=True)
                        nc.scalar.activation(out=h1_sb[:, ec], in_=hps,
                                             func=AF.Relu,
                                             bias=b1_sb[:, h:h + 1])

                    # h2 = relu(W2^T h1 + b2) -> yfm rows 32:64
                    h2ps = ps_a.tile([G, CH], F32, tag="psA")
                    nc.tensor.matmul(h2ps, W2a_sb, h1a[:, ec], start=True, stop=False)
                    nc.tensor.matmul(h2ps, W2b_sb, h1b[:, ec], start=False, stop=True)
                    nc.scalar.activation(out=yfm[G:2 * G, ec], in_=h2ps,
                                         func=AF.Relu, bias=b2_sb)

                    # m = relu(Wmh^T h2 + R_i + bmid) -> yfm rows 0:32
                    mps = ps_a.tile([G, CH], F32, tag="psA")
                    nc.tensor.matmul(mps, Wmh_sb[G:2 * G, :], yfm[G:2 * G, ec],
                                     start=True, stop=False)
                    nc.tensor.matmul(mps, PR_sb[:, 4 * D:4 * D + G],
                                     E_sb[:, ec], start=False, stop=True)
                    nc.scalar.activation(out=yfm[0:G, ec], in_=mps,
                                         func=AF.Relu, bias=bmid_sb)

                # ---------- gate ----------
                # k-major edge order: position e = 128*k + point
                ymean_f = small.tile([2 * G, 128], F32, tag="ymean_f")
                nc.vector.tensor_reduce(
                    out=ymean_f,
                    in_=yfm.rearrange("p (k n) -> p n k", k=K),
                    axis=mybir.AxisListType.X, op=ALU.add)
                ymean = small.tile([128, 128], BF16, tag="ymean")
                nc.vector.tensor_copy(ymean[0:2 * G, :], ymean_f)
                nc.scalar.copy(out=ymean[2 * G:128, :], in_=xT_bf)

                gps = ps_b.tile([128, 128], F32, tag="psB")
                nc.tensor.matmul(gps, Wg_sb, ymean, start=True, stop=True)
                gate_fm = small.tile([128, 128], BF16, tag="gate_fm")
                nc.scalar.activation(out=gate_fm, in_=gps, func=AF.Sigmoid,
                                     bias=bg_sb)
                # gate rows 64:128 again at base partition 0, f32: the gx
                # multiply keeps the big x-channels at full sigmoid precision
                gate_hi = small.tile([D, 128], F32, tag="gate_hi")
                nc.scalar.activation(out=gate_hi, in_=gps[2 * G:128, :],
                                     func=AF.Sigmoid, bias=bg_sb[2 * G:128, :])
                gpm_ps = ps_b.tile([128, 128], BF16, tag="psB")
                nc.tensor.transpose(gpm_ps, gate_fm, id_bf)
                gate_pm = small.tile([128, 128], BF16, tag="gate_pm")
                nc.scalar.copy(out=gate_pm, in_=gpm_ps)

                # gx = gate[64:128] * x   (x-channels of y*gate, constant in k)
                gx_bf = small.tile([D, 128], BF16, tag="gx_bf")
                nc.vector.tensor_mul(gx_bf, gate_hi, xT_sb)
                gxw_ps = ps_b.tile([128, G], F32, tag="psB")
                nc.tensor.matmul(gxw_ps, gx_bf, Wl2_sb,
                                 start=True, stop=True)
                gxw_sb = small.tile([128, G], BF16, tag="gxw")
                nc.scalar.copy(out=gxw_sb, in_=gxw_ps)

                # ---------- gated last layer + max pooling ----------
                # each 512-edge chunk covers 4 neighbor ranks of all 128
                # points; keep a running max across chunks.
                zp_sb = small.tile([G, 128], F32, tag="zp")
                for c in range(NCH):
                    ec = slice(CH * c, CH * (c + 1))
                    ggps = ps_b.tile([2 * G, CH], F32, tag="psB")
                    nc.tensor.matmul(ggps, gate_pm[:, 0:2 * G], E_sb[:, ec],
                                     start=True, stop=True)
                    # yg = (gate broadcast) * yfm — ACT drains psum, the
                    # multiply runs on the otherwise-idle gpsimd (keeps the
                    # DVE free for the top-k scans)
                    gg_sb = small.tile([2 * G, CH], BF16, tag="gg")
                    nc.scalar.copy(out=gg_sb, in_=ggps)
                    yg_sb = small.tile([2 * G, CH], BF16, tag="yg")
                    nc.gpsimd.tensor_tensor(out=yg_sb, in0=gg_sb,
                                            in1=yfm[:, ec], op=ALU.mult)

                    zps = ps_a.tile([G, CH], F32, tag="psA")
                    nc.tensor.matmul(zps, Wl_sb[0:2 * G, :], yg_sb,
                                     start=True, stop=False)
                    nc.tensor.matmul(zps, gxw_sb, E_sb[:, ec],
                                     start=False, stop=True)
                    ztmp = small.tile([G, 128], F32, tag="ztmp")
                    nc.vector.tensor_reduce(
                        out=ztmp,
                        in_=zps.rearrange("p (k n) -> p n k", k=CH // 128),
                        axis=mybir.AxisListType.X, op=ALU.max)
                    if c == 0:
                        nc.vector.tensor_copy(zp_sb, ztmp)
                    else:
                        nc.vector.tensor_tensor(out=zp_sb, in0=zp_sb,
                                                in1=ztmp, op=ALU.max)

                ymax = small.tile([2 * G, 128], BF16, tag="ymax")
                nc.vector.tensor_reduce(
                    out=ymax, in_=yfm.rearrange("p (k n) -> p n k", k=K),
                    axis=mybir.AxisListType.X, op=ALU.max)

                # ---------- assemble output (feature-major, bf16) ----------
                zb_sb = small.tile([G, 128], BF16, tag="zb")
                nc.vector.tensor_add(zb_sb, zp_sb,
                                     blast_sb.to_broadcast([G, 128]))
                yout = small.tile([2 * G, 128], BF16, tag="yout")
                nc.vector.tensor_mul(yout, gate_fm[0:2 * G, :], ymax)

                nc.sync.dma_start(out=out_d[0:G, r0:r0 + 128], in_=zb_sb)
                nc.sync.dma_start(out=out_d[G:3 * G, r0:r0 + 128], in_=yout)
                nc.sync.dma_start(out=out_d[3 * G:COUT, r0:r0 + 128], in_=gx_bf)

    if finalize:
        nc.finalize()   # Bacc.compile: reg alloc, event sems, library loads
    return nc


_NC_CACHE = {}


def _get_nc():
    if "nc" not in _NC_CACHE:
        nc = build_nc()
        # the jit lowering re-serializes the (frozen) program on every
        # call (~60ms for this instruction count); memoize it
        jb = nc.to_json_bytes()
        nc.to_json_bytes = lambda: jb
        _NC_CACHE["nc"] = nc
    return _NC_CACHE["nc"]


def _host_prep(inputs):
    """Shared (replicated) weight-derived arrays, packed per dtype.

    pk16 element offsets: AR@0 (64x288), Bm@18432 (64x256), W2@34816
    (256x32), Wmh@43008 (32x32), Wg@44032 (128x128), Wl@60416 (128x32).
    pk32: Rm is per-core and prepended in make_in_maps; biases follow at
    16384: b1 (128x2), b2@16640, bmid@16672, bg@16704, blast@16832.
    """
    import ml_dtypes
    bf16 = ml_dtypes.bfloat16
    W1 = np.asarray(inputs["W1"], np.float32)
    Wmid = np.asarray(inputs["Wmid"], np.float32)
    A = W1[0:D] - W1[2 * D:3 * D]
    Bm = W1[D:2 * D] + W1[2 * D:3 * D]
    AR = np.concatenate([A, Wmid[G:G + D]], axis=1)          # (64, 288)
    Wg_adj = np.asarray(inputs["Wg"], np.float32).copy()
    Wg_adj[0:2 * G] /= K
    pk16 = np.concatenate([
        np.ascontiguousarray(AR).astype(bf16).ravel(),
        np.ascontiguousarray(Bm).astype(bf16).ravel(),
        np.asarray(inputs["W2"], np.float32).astype(bf16).ravel(),
        np.ascontiguousarray(Wmid[0:G]).astype(bf16).ravel(),
        Wg_adj.astype(bf16).ravel(),
        np.asarray(inputs["Wlast"], np.float32).astype(bf16).ravel(),
    ])
    assert pk16.size == 64512
    bias32 = np.concatenate([
        np.ascontiguousarray(
            np.asarray(inputs["b1"], np.float32).reshape(2, 128).T).ravel(),
        np.asarray(inputs["b2"], np.float32).ravel(),
        np.asarray(inputs["bmid"], np.float32).ravel(),
        np.asarray(inputs["bg"], np.float32).ravel(),
        np.asarray(inputs["blast"], np.float32).ravel(),
    ]).astype(np.float32)
    return pk16, bias32


def make_in_maps(inputs):
    import ml_dtypes
    x = np.asarray(inputs["x"], np.float32)
    pos = np.asarray(inputs["pos"], np.float32)
    pk16, bias32 = _host_prep(inputs)
    in_maps = []
    for c in range(B):
        p = pos[c]
        sq = (p * p).sum(-1)
        R = np.concatenate([p.T, sq[None, :]], axis=0).astype(np.float32)
        in_maps.append({
            "x": np.ascontiguousarray(x[c]).astype(ml_dtypes.bfloat16),
            "pk16": pk16,
            "pk32": np.concatenate([R.ravel(), bias32]),
        })
    return in_maps


def kernel(**inputs) -> np.ndarray:
    nc = _get_nc()
    in_maps = make_in_maps(inputs)
    res = run_bass_kernel_spmd(nc, in_maps, list(range(B)))
    # out is feature-major bf16 [COUT, N]; transpose + upcast on host
    return np.stack([
        np.asarray(res.results[c]["out"]).T.astype(np.float32) for c in range(B)
    ])


if __name__ == "__main__":
    nc = build_nc()
    print("built ok:",
          sum(len(bb.instructions) for bb in nc.main_func.blocks), "instructions")
